# revision 13
# baseline (speedup 1.0000x reference)
"""BiomarkerGNN Trainium2 kernel: 3-layer GCN + MLP branch + gated fusion heads.

Strategy (8 NeuronCores, node-sharded):
  - Core c owns nodes [c*12500, (c+1)*12500).
  - GCN norm is separable: norm[e] = dinv[src]*dinv[dst]. Per layer each core
    computes hws = dinv * (h @ W) for its own nodes (dense matmuls), then an
    AllGather assembles the full [100k, 64] fp32 table in each core's HBM.
  - Edge aggregation per core: dma_gather fetches hws[src] rows (256B) for the
    core's edges; a one-hot matmul scatter-adds them into a PSUM tile per
    128-dst-node tile: S[e, d] = dinv[dst[e]] * 1{dstl[e]==d}, built in one
    dual-op tensor_scalar (iota == dstl) * dinvd.
  - Self-loops are applied as a per-tile diagonal matmul over the core's own
    hws rows (no gather).
  - Sources are bucketed into 5 ranges of 20000 so gather indices fit int16.
  - The epilogue relu(agg + b) runs on ScalarE straight out of PSUM
    (feature-major) and feeds the next layer's dense matmul with no transposes.

Self-contained: hardcodes all shapes. Host numpy only prepares index metadata,
sharding, and folds BatchNorm constants.
"""

import numpy as np

import concourse.bass as bass
import concourse.mybir as mybir
import concourse.bacc as bacc
from concourse import tile
from concourse import bass_utils

N = 100000
F = 128
H = 64
PJ = 32
NCORES = 8
NPC = N // NCORES            # 12500 nodes per core
NT = (NPC + 127) // 128      # 98 dst tiles per core
NTPAD = NT * 128             # 12544
NB = 5                       # source buckets
BUCKET = N // NB             # 20000
CALL_CHUNKS = 16             # chunks (of 128 idx) per dma_gather call
BN_EPS = 1e-5
DT = mybir.dt


# ---------------------------------------------------------------- host prep

def _wrap_idx_calls(flat_vals, n_chunks):
    """Pack a bucket stream's gather indices into the SWDGE int16 layout:
    per call a [128, cc*8] block (position j at [j%16, j//16], replicated
    across the 8 Q7 16-partition groups)."""
    blocks = []
    for c0 in range(0, n_chunks, CALL_CHUNKS):
        cc = min(CALL_CHUNKS, n_chunks - c0)
        v = flat_vals[c0 * 128:(c0 + cc) * 128]
        cols = (cc * 128) // 16
        blk = v.reshape(cols, 16).T.astype(np.int16)
        blocks.append(np.tile(blk, (8, 1)))
    return np.concatenate(blocks, axis=1) if blocks else np.zeros((128, 0), np.int16)


def _prep(x, edge_index):
    src_e = np.asarray(edge_index[0], dtype=np.int64)
    dst_e = np.asarray(edge_index[1], dtype=np.int64)

    deg = (np.bincount(dst_e, minlength=N) + 1).astype(np.float64)  # + self loop
    dinv = (1.0 / np.sqrt(deg)).astype(np.float32)

    per_core_raw = []
    cnts = np.zeros((NCORES, NT * NB), dtype=np.int64)
    for c in range(NCORES):
        sel = (dst_e >= c * NPC) & (dst_e < (c + 1) * NPC)
        s = src_e[sel]
        d = dst_e[sel] - c * NPC
        t = d >> 7
        b = s // BUCKET
        cell = t * NB + b
        order = np.lexsort((s, cell))
        s, d, cell, b = s[order], d[order], cell[order], b[order]
        cnts[c] = np.bincount(cell, minlength=NT * NB)
        per_core_raw.append((s, d, cell, b))

    CH = -(-cnts.max(axis=0) // 128)          # chunks per cell (shared)
    NCH = int(CH.sum())
    ccs = np.zeros(NT * NB + 1, dtype=np.int64)
    np.cumsum(CH, out=ccs[1:])

    gchunks_b = []
    for b in range(NB):
        parts = [np.arange(ccs[t * NB + b], ccs[t * NB + b] + CH[t * NB + b])
                 for t in range(NT)]
        gchunks_b.append(np.concatenate(parts))
    pos_in_bucket = np.zeros(NCH, dtype=np.int64)
    for b in range(NB):
        pos_in_bucket[gchunks_b[b]] = np.arange(len(gchunks_b[b]))

    inputs = []
    for c in range(NCORES):
        s, d, cell, b = per_core_raw[c]
        cell_edge_start = np.zeros(NT * NB + 1, dtype=np.int64)
        np.cumsum(cnts[c], out=cell_edge_start[1:])
        rank = np.arange(len(s)) - cell_edge_start[cell]
        pos = ccs[cell] * 128 + rank

        idx_full = np.zeros(NCH * 128, dtype=np.int64)
        dstl_full = np.full(NCH * 128, -1.0, dtype=np.float32)
        dinvd_full = np.zeros(NCH * 128, dtype=np.float32)
        idx_full[pos] = s - b * BUCKET
        dstl_full[pos] = (d & 127).astype(np.float32)
        dinvd_full[pos] = dinv[d + c * NPC]

        idx_mats = {}
        for bb in range(NB):
            flat = idx_full.reshape(NCH, 128)[gchunks_b[bb]].reshape(-1)
            idx_mats[bb] = _wrap_idx_calls(flat, len(gchunks_b[bb]))

        xT = np.zeros((F, NTPAD), dtype=np.float32)
        xT[:, :NPC] = x[c * NPC:(c + 1) * NPC].T
        dv = np.zeros(NTPAD, dtype=np.float32)
        dv[:NPC] = dinv[c * NPC:(c + 1) * NPC]
        dinv_pp = dv.reshape(NT, 128).T.copy()

        inputs.append(dict(
            xT=xT,
            dstl=dstl_full.reshape(NCH, 128).T.copy(),
            dinvd=dinvd_full.reshape(NCH, 128).T.copy(),
            dinv_pp=dinv_pp,
            **{f"idx{bb}": idx_mats[bb] for bb in range(NB)},
        ))

    struct = dict(CH=CH, NCH=NCH, ccs=ccs, pos_in_bucket=pos_in_bucket,
                  CB=[len(gchunks_b[b]) for b in range(NB)])
    return inputs, struct, dinv


# ---------------------------------------------------------------- program

def _build_program(struct, wdict):
    CH = struct["CH"]
    NCH = struct["NCH"]
    ccs = struct["ccs"]
    pos_in_bucket = struct["pos_in_bucket"]
    CB = struct["CB"]

    nc = bacc.Bacc("TRN2", target_bir_lowering=False, debug=False,
                   enable_asserts=True, num_devices=NCORES)

    t_xT = nc.dram_tensor("xT", [F, NTPAD], DT.float32, kind="ExternalInput")
    t_idx = [nc.dram_tensor(f"idx{b}", [128, CB[b] * 8], DT.int16,
                            kind="ExternalInput") for b in range(NB)]
    t_dstl = nc.dram_tensor("dstl", [128, NCH], DT.float32, kind="ExternalInput")
    t_dinvd = nc.dram_tensor("dinvd", [128, NCH], DT.float32, kind="ExternalInput")
    t_dinvpp = nc.dram_tensor("dinv_pp", [128, NT], DT.float32, kind="ExternalInput")
    t_iota = nc.dram_tensor("iota", [128, 128], DT.float32, kind="ExternalInput")
    t_iotac = nc.dram_tensor("iotac", [128, 1], DT.float32, kind="ExternalInput")
    t_w = {}
    for name, arr in wdict.items():
        if np.isscalar(arr):
            continue
        t_w[name] = nc.dram_tensor(name, list(arr.shape), DT.float32,
                                   kind="ExternalInput")
    gateb = float(wdict["gateb_s"])
    cb = float(wdict["cb_s"])

    t_logits = nc.dram_tensor("logits_o", [1, NTPAD], DT.float32,
                              kind="ExternalOutput")
    t_zproj = nc.dram_tensor("zproj_o", [PJ, NTPAD], DT.float32,
                             kind="ExternalOutput")

    t_agin = [nc.dram_tensor(f"ag_in{l}", [NPC, H], DT.float32, kind="Internal")
              for l in range(3)]
    t_table = [nc.dram_tensor(f"table{l}", [N, H], DT.float32, kind="Internal")
               for l in range(3)]
    t_hml = nc.dram_tensor("hml_d", [H, NTPAD], DT.float32, kind="Internal")

    RG = [list(range(NCORES))]
    AF = mybir.ActivationFunctionType
    OP = mybir.AluOpType

    with tile.TileContext(nc) as tc:
        with tc.tile_pool(name="const", bufs=1) as cp, \
             tc.tile_pool(name="stage", bufs=2) as stp, \
             tc.tile_pool(name="spool", bufs=4) as sp, \
             tc.tile_pool(name="work", bufs=3) as wp:

            idx_sb = []
            for b in range(NB):
                tsb = cp.tile([128, CB[b] * 8], DT.int16, tag=f"idx{b}",
                              name=f"idxsb{b}")
                nc.sync.dma_start(tsb[:], t_idx[b][:, :])
                idx_sb.append(tsb)
            dstl_sb = cp.tile([128, NCH], DT.float32, tag="dstl")
            nc.sync.dma_start(dstl_sb[:], t_dstl[:, :])
            dinvd_sb = cp.tile([128, NCH], DT.float32, tag="dinvd")
            nc.sync.dma_start(dinvd_sb[:], t_dinvd[:, :])
            dinvpp_sb = cp.tile([128, NT], DT.float32, tag="dinvpp")
            nc.sync.dma_start(dinvpp_sb[:], t_dinvpp[:, :])
            iota_sb = cp.tile([128, 128], DT.float32, tag="iota")
            nc.sync.dma_start(iota_sb[:], t_iota[:, :])
            iotac_sb = cp.tile([128, 1], DT.float32, tag="iotac")
            nc.sync.dma_start(iotac_sb[:], t_iotac[:, :])

            w_sb = {}
            for name in t_w:
                shp = list(wdict[name].shape)
                w_sb[name] = cp.tile(shp, DT.float32, tag=name, name=f"w_{name}")
                nc.sync.dma_start(w_sb[name][:], t_w[name][(slice(None),) * len(shp)])
            ones_sb = cp.tile([1, H], DT.float32, tag="ones")
            nc.vector.memset(ones_sb[:], 1.0)

            hT = cp.tile([H, NTPAD], DT.float32, tag="hT")   # current GCN act

            with tc.tile_pool(name="psq", bufs=3, space="PSUM") as psq, \
                 tc.tile_pool(name="psd", bufs=2, space="PSUM") as psd:
                # ============ layer 0 dense: table for GCN l1 + MLP branch
                for t in range(NT):
                    vt = min(128, NPC - t * 128)
                    xt = wp.tile([F, 128], DT.float32, tag="xt", name=f"xt{t}")
                    nc.sync.dma_start(xt[:], t_xT[:, t * 128:(t + 1) * 128])
                    pd = psd.tile([128, H], DT.float32, tag="d", name=f"pd0_{t}")
                    nc.tensor.matmul(pd[:], xt[:], w_sb["gW0"][:],
                                     start=True, stop=True)
                    hws = wp.tile([128, H], DT.float32, tag="hws", name=f"hws0_{t}")
                    nc.vector.tensor_scalar(hws[:], pd[:],
                                            dinvpp_sb[:, t:t + 1], None, OP.mult)
                    nc.sync.dma_start(t_agin[0][t * 128:t * 128 + vt, :], hws[:vt, :])
                    pq = psq.tile([H, 128], DT.float32, tag="q", name=f"pm0_{t}")
                    nc.tensor.matmul(pq[:], w_sb["W1f"][:], xt[:],
                                     start=True, stop=True)
                    hmo = wp.tile([H, 128], DT.float32, tag="hmo", name=f"hmo{t}")
                    nc.scalar.activation(hmo[:], pq[:], AF.Relu, bias=w_sb["t1"][:])
                    nc.sync.dma_start(t_hml[:, t * 128:(t + 1) * 128], hmo[:])

                nc.gpsimd.collective_compute(
                    "AllGather", OP.bypass, replica_groups=RG,
                    ins=[t_agin[0][:, :]], outs=[t_table[0][:, :]])

                # ============ GCN layers
                for l in range(3):
                    gb_tile = w_sb[f"gb{l}"]
                    stage_tiles = {}

                    def get_stage(b, call, _l=l):
                        key = (b, call)
                        if key not in stage_tiles:
                            cc = min(CALL_CHUNKS, CB[b] - call * CALL_CHUNKS)
                            st = stp.tile([128, CALL_CHUNKS * H], DT.float32,
                                          tag=f"st{b}", name=f"st{_l}_{b}_{call}")
                            nidx = cc * 128
                            out_ap = st[:].rearrange("p (c e) -> p c e", e=H)[:, :cc, :]
                            nc.gpsimd.dma_gather(
                                out_ap,
                                t_table[_l][b * BUCKET:(b + 1) * BUCKET, :],
                                idx_sb[b][:, call * CALL_CHUNKS * 8:
                                          call * CALL_CHUNKS * 8 + cc * 8],
                                nidx, nidx, H, queue_num=0,
                                single_packet=False)
                            stage_tiles[key] = st
                        return stage_tiles[key]

                    for t in range(NT):
                        vt = min(128, NPC - t * 128)
                        # self-loop operand: own hws rows for this tile
                        selfw = wp.tile([128, H], DT.float32, tag="selfw",
                                        name=f"selfw{l}_{t}")
                        if vt < 128:
                            nc.vector.memset(selfw[:], 0.0)
                        nc.sync.dma_start(selfw[:vt, :],
                                          t_agin[l][t * 128:t * 128 + vt, :])
                        sdg = sp.tile([128, 128], DT.float32, tag="s",
                                      name=f"sdg{l}_{t}")
                        nc.vector.tensor_scalar(sdg[:], iota_sb[:], iotac_sb[:],
                                                dinvpp_sb[:, t:t + 1],
                                                OP.is_equal, OP.mult)
                        pq = psq.tile([H, 128], DT.float32, tag="q",
                                      name=f"pq{l}_{t}")
                        ntot = 1 + sum(int(CH[t * NB + b]) for b in range(NB))
                        nc.tensor.matmul(pq[:], selfw[:], sdg[:],
                                         start=True, stop=(ntot == 1))
                        done = 1
                        for b in range(NB):
                            for k in range(int(CH[t * NB + b])):
                                g = int(ccs[t * NB + b]) + k
                                pb = int(pos_in_bucket[g])
                                call, slot = divmod(pb, CALL_CHUNKS)
                                st = get_stage(b, call)
                                s_t = sp.tile([128, 128], DT.float32, tag="s",
                                              name=f"s{l}_{g}")
                                nc.vector.tensor_scalar(
                                    s_t[:], iota_sb[:],
                                    dstl_sb[:, g:g + 1], dinvd_sb[:, g:g + 1],
                                    OP.is_equal, OP.mult)
                                nc.tensor.matmul(
                                    pq[:], st[:, slot * H:(slot + 1) * H], s_t[:],
                                    start=False, stop=(done == ntot - 1))
                                done += 1
                        nc.scalar.activation(hT[:, t * 128:(t + 1) * 128], pq[:],
                                             AF.Relu, bias=gb_tile[:])
                        if l < 2:
                            pd = psd.tile([128, H], DT.float32, tag="d",
                                          name=f"pd{l}_{t}")
                            nc.tensor.matmul(pd[:], hT[:, t * 128:(t + 1) * 128],
                                             w_sb[f"gW{l + 1}"][:],
                                             start=True, stop=True)
                            hws = wp.tile([128, H], DT.float32, tag="hws",
                                          name=f"hws{l}_{t}")
                            nc.vector.tensor_scalar(hws[:], pd[:],
                                                    dinvpp_sb[:, t:t + 1], None,
                                                    OP.mult)
                            nc.sync.dma_start(
                                t_agin[l + 1][t * 128:t * 128 + vt, :], hws[:vt, :])
                    if l < 2:
                        nc.gpsimd.collective_compute(
                            "AllGather", OP.bypass, replica_groups=RG,
                            ins=[t_agin[l + 1][:, :]], outs=[t_table[l + 1][:, :]])

            # ============ final heads (hT holds hg)
            with tc.tile_pool(name="psg", bufs=2, space="PSUM") as psg, \
                 tc.tile_pool(name="psbc", bufs=2, space="PSUM") as psbc, \
                 tc.tile_pool(name="psp", bufs=2, space="PSUM") as psp, \
                 tc.tile_pool(name="psz", bufs=1, space="PSUM") as psz, \
                 tc.tile_pool(name="psl", bufs=1, space="PSUM") as psl:
                for j in range(0, NTPAD, 512):
                    w = min(512, NTPAD - j)
                    cols = slice(j, j + w)
                    hm_t = wp.tile([H, 512], DT.float32, tag="hmin", bufs=2,
                                   name=f"hmin{j}")
                    nc.sync.dma_start(hm_t[:, :w], t_hml[:, cols])
                    pg = psg.tile([1, 512], DT.float32, tag="g", name=f"pg{j}")
                    nc.tensor.matmul(pg[:, :w], w_sb["gA"][:], hm_t[:, :w],
                                     start=True, stop=False)
                    nc.tensor.matmul(pg[:, :w], w_sb["gB"][:], hT[:, cols],
                                     start=False, stop=True)
                    sg = wp.tile([1, 512], DT.float32, tag="sg", bufs=2,
                                 name=f"sg{j}")
                    nc.scalar.activation(sg[:, :w], pg[:, :w], AF.Sigmoid,
                                         bias=gateb)
                    pbc = psbc.tile([H, 512], DT.float32, tag="bc", name=f"pbc{j}")
                    nc.tensor.matmul(pbc[:, :w], ones_sb[:], sg[:, :w],
                                     start=True, stop=True)
                    dif = wp.tile([H, 512], DT.float32, tag="dif", bufs=2,
                                  name=f"dif{j}")
                    nc.vector.tensor_tensor(dif[:, :w], hT[:, cols], hm_t[:, :w],
                                            OP.subtract)
                    fus = wp.tile([H, 512], DT.float32, tag="fus", bufs=2,
                                  name=f"fus{j}")
                    nc.vector.tensor_tensor(fus[:, :w], dif[:, :w], pbc[:, :w],
                                            OP.mult)
                    nc.vector.tensor_tensor(fus[:, :w], fus[:, :w], hm_t[:, :w],
                                            OP.add)
                    pp1 = psp.tile([H, 512], DT.float32, tag="p1", name=f"pp1{j}")
                    nc.tensor.matmul(pp1[:, :w], w_sb["pW1"][:], fus[:, :w],
                                     start=True, stop=True)
                    z1 = wp.tile([H, 512], DT.float32, tag="z1", bufs=2,
                                 name=f"z1{j}")
                    nc.scalar.activation(z1[:, :w], pp1[:, :w], AF.Relu,
                                         bias=w_sb["pb1"][:])
                    pz = psz.tile([PJ, 512], DT.float32, tag="z2", name=f"pz{j}")
                    nc.tensor.matmul(pz[:, :w], w_sb["pW2"][:], z1[:, :w],
                                     start=True, stop=True)
                    zo = wp.tile([PJ, 512], DT.float32, tag="zo", bufs=2,
                                 name=f"zo{j}")
                    nc.scalar.activation(zo[:, :w], pz[:, :w], AF.Identity,
                                         bias=w_sb["pb2"][:])
                    nc.sync.dma_start(t_zproj[:, cols], zo[:, :w])
                    pl = psl.tile([1, 512], DT.float32, tag="lg", name=f"pl{j}")
                    nc.tensor.matmul(pl[:, :w], w_sb["cW"][:], fus[:, :w],
                                     start=True, stop=True)
                    lgo = wp.tile([1, 512], DT.float32, tag="lgo", bufs=2,
                                  name=f"lgo{j}")
                    nc.scalar.activation(lgo[:, :w], pl[:, :w], AF.Identity,
                                         bias=cb)
                    nc.sync.dma_start(t_logits[:, cols], lgo[:, :w])

    nc.compile()
    return nc


# ---------------------------------------------------------------- runner

def run(inputs, trace=False):
    x = np.asarray(inputs["x"], dtype=np.float32)
    edge_index = np.asarray(inputs["edge_index"])

    per_core, struct, _dinv = _prep(x, edge_index)

    bnscale = (np.asarray(inputs["gamma"]) /
               np.sqrt(np.asarray(inputs["rvar"]) + BN_EPS)).astype(np.float32)
    bnshift = (np.asarray(inputs["beta"]) -
               np.asarray(inputs["rmean"]) * bnscale).astype(np.float32)
    W1f = (np.asarray(inputs["W1"]) * bnscale[None, :]).astype(np.float32)
    t1 = (np.asarray(inputs["b1"]) * bnscale + bnshift).astype(np.float32)

    gateW = np.asarray(inputs["gateW"], dtype=np.float32)
    wdict = dict(
        gW0=np.asarray(inputs["gW0"], dtype=np.float32),
        gW1=np.asarray(inputs["gW1"], dtype=np.float32),
        gW2=np.asarray(inputs["gW2"], dtype=np.float32),
        W1f=W1f,
        t1=t1.reshape(H, 1),
        gb0=np.asarray(inputs["gb0"], dtype=np.float32).reshape(H, 1),
        gb1=np.asarray(inputs["gb1"], dtype=np.float32).reshape(H, 1),
        gb2=np.asarray(inputs["gb2"], dtype=np.float32).reshape(H, 1),
        gA=gateW[:H].reshape(H, 1).copy(),
        gB=gateW[H:].reshape(H, 1).copy(),
        pW1=np.asarray(inputs["pW1"], dtype=np.float32),
        pb1=np.asarray(inputs["pb1"], dtype=np.float32).reshape(H, 1),
        pW2=np.asarray(inputs["pW2"], dtype=np.float32),
        pb2=np.asarray(inputs["pb2"], dtype=np.float32).reshape(PJ, 1),
        cW=np.asarray(inputs["cW"], dtype=np.float32).reshape(H, 1),
        gateb_s=float(np.asarray(inputs["gateb"]).reshape(-1)[0]),
        cb_s=float(np.asarray(inputs["cb"]).reshape(-1)[0]),
    )

    nc = _build_program(struct, wdict)

    iota = np.broadcast_to(np.arange(128, dtype=np.float32), (128, 128)).copy()
    iotac = np.arange(128, dtype=np.float32).reshape(128, 1).copy()
    in_maps = []
    for c in range(NCORES):
        m = dict(per_core[c])
        m["iota"] = iota
        m["iotac"] = iotac
        for name, arr in wdict.items():
            if not np.isscalar(arr):
                m[name] = arr
        in_maps.append(m)

    res = bass_utils.run_bass_kernel_spmd(
        nc, in_maps, core_ids=list(range(NCORES)), trace=trace)

    logits = np.concatenate(
        [res.results[c]["logits_o"][0, :NPC] for c in range(NCORES)])
    z_proj = np.concatenate(
        [res.results[c]["zproj_o"][:, :NPC].T for c in range(NCORES)], axis=0)
    return (logits.astype(np.float32), np.ascontiguousarray(z_proj, dtype=np.float32)), res


def kernel(**inputs):
    out, _ = run(inputs, trace=False)
    return out
